# revision 8
# baseline (speedup 1.0000x reference)
"""Two-NEFF Trainium2 kernel for fused BatchNorm1d(train) -> Linear -> ELU.

  y = ELU( ((x - mean) * gamma.rsqrt(var+eps) + beta) @ W.T )

Data-parallel over 8 cores (131072 rows each); the 2KB cross-core stat
reduction plus the 256-element stat finalization run on the HOST between
two NEFF launches (measured cheaper than an on-device collective).

  NEFF A (per core): stream x (f32), ACT-downcast to bf16, PE-transpose
      128x128 bf16 blocks (1 cyc/row) into PSUM, DVE evacuation with fused
      accum_out (per-feature sum) + DVE square with fused accum_out (sum of
      squares), stage x^T as two [128, N] bf16 DRAM tensors with plain
      2KB-line DMAs. Staged columns are (j,p)-permuted within each
      1024-column tile (input tiling is n = t*1024 + p*8 + j); phase C is
      column-independent and the host un-permutes for free.
  host: reduce the 8x[128,4] partial stats, finalize
      s = gamma*rsqrt(var+eps), t = beta - mean*s  (float64).
  NEFF C (per core): fold s into W (per-partition scale of W.T), bias
      b' = t @ W.T via tiny f32 matmuls; stream x^T bf16 with contiguous
      DMAs, W-stationary bf16 matmuls -> y^T in PSUM (output features on
      partitions, so b' is a per-partition bias); ELU via
      ELU(z) = max(z, min(exp(z)-1, 0)): ACT Exp (fused bias) + ONE fused
      custom-DVE op (ELU_TAIL_ANT, registered below); write y^T bf16.
  host: transpose y^T -> y and upcast to f32 (jax cpu).
"""

import functools
import sys

import numpy as np

if "/opt/trn_rl_repo" not in sys.path:
    sys.path.insert(0, "/opt/trn_rl_repo")

N_TOTAL = 1048576
F = 256
NCORES = 8
N_SHARD = N_TOTAL // NCORES
P = 128
RT = 8  # row-blocks per phase-A tile (1024 rows)
EPS = 1e-5


def _register_elu_tail():
    """Register the fused ELU-tail custom DVE op (idempotent).

    out = max(in0 + s0, min(in1 - 1, 0))   [in0 = z-b from PSUM, in1 = exp(z),
                                            s0 = per-partition bias b]
    """
    import concourse.dve_ops as DO
    from concourse.dve_spec import C0, One, Spec, Src0, Src1, Zero, lower, maxx, minn
    from concourse.dve_uop import DveOpSpec

    name = "ELU_TAIL_ANT"
    if name in DO._SUB_OPCODE_FOR_NAME:
        return next(op for op in DO.OPS if op.name == name)
    spec = Spec(
        body=maxx(Src0 + C0, minn(Src1 - One, Zero)),
        reference=lambda in0, in1, s0, s1, imm2: np.maximum(
            in0 + s0, np.minimum(in1 - 1.0, 0.0)
        ),
    )
    shas = {}
    for ver in ("v3", "v4"):
        ds = DveOpSpec(
            name=name, opcode=0, uops=lower(spec, ver=ver),
            rd1_en=DO.has_src1(spec),
        )
        shas[ver] = ds.sha(ver)
    op = DO.DveOp(name, spec, subdim=False, uops_sha=shas)
    DO.OPS.append(op)
    DO.CUSTOM_DVE_SPECS[op.name] = op.spec
    DO._SUB_OPCODE_FOR_NAME[op.name] = DO._CUSTOM_DVE_ROW_BASE + len(DO.OPS) - 1
    return op


def _bass(ncores):
    from concourse import bacc

    return bacc.Bacc(
        "TRN2", target_bir_lowering=False, debug=False, num_devices=ncores
    )


def build_a(n_shard=N_SHARD, ncores=NCORES, rt=RT, repeat=1):
    """Phase A: bf16 transpose-staging + per-feature stats.

    Outputs: xt0, xt1 [P, n_shard] bf16 (x^T feature halves),
             st [P, 4] f32 = [sum_h0 | sum_h1 | sumsq_h0 | sumsq_h1].
    """
    import concourse.tile as tile
    from concourse import mybir

    f32 = mybir.dt.float32
    bf16 = mybir.dt.bfloat16
    AF = mybir.ActivationFunctionType
    OP = mybir.AluOpType

    nc = _bass(ncores)
    x = nc.dram_tensor("x", [n_shard, F], f32, kind="ExternalInput").ap()
    xt0 = nc.dram_tensor("xt0", [P, n_shard], bf16, kind="ExternalOutput").ap()
    xt1 = nc.dram_tensor("xt1", [P, n_shard], bf16, kind="ExternalOutput").ap()
    st = nc.dram_tensor("st", [P, 4], f32, kind="ExternalOutput").ap()

    R = P * rt  # rows per tile
    T = n_shard // R

    with tile.TileContext(nc) as tc:
        with tc.tile_pool(name="wp", bufs=1) as wp:
            # identity for PE transposes: ident[i,j] = (j == i)
            col = wp.tile([P, P], f32)
            nc.gpsimd.iota(col[:], pattern=[[1, P]], base=0, channel_multiplier=0,
                           allow_small_or_imprecise_dtypes=True)
            row = wp.tile([P, 1], f32)
            nc.gpsimd.iota(row[:], pattern=[[0, 1]], base=0, channel_multiplier=1,
                           allow_small_or_imprecise_dtypes=True)
            ident = wp.tile([P, P], bf16)
            nc.vector.tensor_scalar(ident[:], col[:], row[:], None, OP.is_equal)

            for _rep in range(repeat):
                with tc.tile_pool(name="ac", bufs=1) as ac:
                    sumc0 = ac.tile([P, T], f32)
                    sumc1 = ac.tile([P, T], f32)
                    sqc0 = ac.tile([P, T], f32)
                    sqc1 = ac.tile([P, T], f32)
                    with tc.tile_pool(name="sa", bufs=3) as sa, tc.tile_pool(
                        name="psA", bufs=2, space="PSUM"
                    ) as psA:
                        xv = x.rearrange("(t p j) f -> t p j f", p=P, j=rt)
                        for t in range(T):
                            xt = sa.tile([P, rt, F], f32, tag="xt")
                            nc.sync.dma_start(xt[:], xv[t])
                            xb = sa.tile([P, rt, F], bf16, tag="xb")
                            nc.vector.tensor_copy(xb[:], xt[:])
                            ps0 = psA.tile([P, rt, P], bf16, tag="ps0")
                            ps1 = psA.tile([P, rt, P], bf16, tag="ps1")
                            for j in range(rt):
                                nc.tensor.transpose(
                                    ps0[:, j, :], xb[:, j, 0:P], ident[:]
                                )
                                nc.tensor.transpose(
                                    ps1[:, j, :], xb[:, j, P:F], ident[:]
                                )
                            # evacuate PSUM -> SBUF (ACT) with fused column sums
                            xs0 = sa.tile([P, rt * P], bf16, tag="xs0")
                            nc.scalar.activation(
                                xs0[:], ps0[:].rearrange("p j c -> p (j c)"),
                                AF.Identity, accum_out=sumc0[:, t : t + 1],
                            )
                            xs1 = sa.tile([P, rt * P], bf16, tag="xs1")
                            nc.scalar.activation(
                                xs1[:], ps1[:].rearrange("p j c -> p (j c)"),
                                AF.Identity, accum_out=sumc1[:, t : t + 1],
                            )
                            # sum of squares on DVE with fused accumulate
                            sq0 = sa.tile([P, rt * P], bf16, tag="sq0")
                            nc.vector.scalar_tensor_tensor(
                                sq0[:], xs0[:], 0.0, xs0[:], OP.add, OP.mult,
                                accum_out=sqc0[:, t : t + 1],
                            )
                            sq1 = sa.tile([P, rt * P], bf16, tag="sq1")
                            nc.vector.scalar_tensor_tensor(
                                sq1[:], xs1[:], 0.0, xs1[:], OP.add, OP.mult,
                                accum_out=sqc1[:, t : t + 1],
                            )
                            nc.sync.dma_start(xt0[:, t * R : (t + 1) * R], xs0[:])
                            nc.sync.dma_start(xt1[:, t * R : (t + 1) * R], xs1[:])
                    st_sb = ac.tile([P, 4], f32)
                    nc.vector.tensor_reduce(
                        st_sb[:, 0:1], sumc0[:], mybir.AxisListType.X, OP.add
                    )
                    nc.vector.tensor_reduce(
                        st_sb[:, 1:2], sumc1[:], mybir.AxisListType.X, OP.add
                    )
                    nc.vector.tensor_reduce(
                        st_sb[:, 2:3], sqc0[:], mybir.AxisListType.X, OP.add
                    )
                    nc.vector.tensor_reduce(
                        st_sb[:, 3:4], sqc1[:], mybir.AxisListType.X, OP.add
                    )
                    nc.sync.dma_start(st, st_sb[:])
    nc.compile()
    return nc


def build_c(n_shard=N_SHARD, ncores=NCORES, repeat=1):
    """Phase C: y^T = ELU(Wp @ x^T + b') with W stationary on the PE.

    Inputs: xt0, xt1 [P, n_shard] bf16; wt [F, F] f32 (= W.T);
            sg [P, 4] f32 = [s_h0 | s_h1 | t_h0 | t_h1] (host-finalized).
    Outputs: yt0, yt1 [P, n_shard] bf16 (y^T output-feature halves).
    """
    import concourse.tile as tile
    from concourse import mybir

    elu_op = _register_elu_tail()

    f32 = mybir.dt.float32
    bf16 = mybir.dt.bfloat16
    AF = mybir.ActivationFunctionType
    OP = mybir.AluOpType

    nc = _bass(ncores)
    xt0 = nc.dram_tensor("xt0", [P, n_shard], bf16, kind="ExternalInput").ap()
    xt1 = nc.dram_tensor("xt1", [P, n_shard], bf16, kind="ExternalInput").ap()
    wt = nc.dram_tensor("wt", [F, F], f32, kind="ExternalInput").ap()
    sg = nc.dram_tensor("sg", [P, 4], f32, kind="ExternalInput").ap()
    yt0 = nc.dram_tensor("yt0", [P, n_shard], bf16, kind="ExternalOutput").ap()
    yt1 = nc.dram_tensor("yt1", [P, n_shard], bf16, kind="ExternalOutput").ap()

    CT = 4096  # columns (rows of x) per main-loop tile
    NCH = CT // 512  # 512-column PSUM chunks
    T = n_shard // CT

    with tile.TileContext(nc) as tc:
        with tc.tile_pool(name="wp", bufs=1) as wp:
            wt_sb = wp.tile([P, 2, F], f32)
            nc.sync.dma_start(wt_sb[:], wt.rearrange("(c p) f -> p c f", p=P))
            sg_sb = wp.tile([P, 4], f32)
            nc.sync.dma_start(sg_sb[:], sg)

            for _rep in range(repeat):
                with tc.tile_pool(name="pb", bufs=1, space="PSUM") as psB:
                    # Wp^T[f, o] = W^T[f, o] * s[f]  (per-partition scale)
                    wpT = wp.tile([P, 2, F], bf16)
                    for c in range(2):
                        nc.vector.tensor_scalar(
                            wpT[:, c, :], wt_sb[:, c, :], sg_sb[:, c : c + 1],
                            None, OP.mult,
                        )
                    # b'[o] = sum_f t[f] * W^T[f, o], as two [128,1] columns
                    bps = psB.tile([P, 2], f32, tag="bps")
                    for ob in range(2):
                        for c in range(2):
                            nc.tensor.matmul(
                                bps[:, ob : ob + 1],
                                wt_sb[:, c, ob * P : (ob + 1) * P],
                                sg_sb[:, 2 + c : 3 + c],
                                start=(c == 0),
                                stop=(c == 1),
                            )
                    b_sb = wp.tile([P, 2], f32)
                    nc.vector.tensor_copy(b_sb[:], bps[:])

                with tc.tile_pool(name="cp", bufs=2) as cp, tc.tile_pool(
                    name="ep", bufs=2
                ) as ep, tc.tile_pool(name="psC", bufs=2, space="PSUM") as psC:
                    for t in range(T):
                        x0 = cp.tile([P, CT], bf16, tag="x0")
                        nc.sync.dma_start(x0[:], xt0[:, t * CT : (t + 1) * CT])
                        x1 = cp.tile([P, CT], bf16, tag="x1")
                        nc.sync.dma_start(x1[:], xt1[:, t * CT : (t + 1) * CT])
                        y0 = cp.tile([P, CT], bf16, tag="y0")
                        y1 = cp.tile([P, CT], bf16, tag="y1")
                        for ch in range(NCH):
                            sl = slice(ch * 512, (ch + 1) * 512)
                            for ob, yo in ((0, y0), (1, y1)):
                                ps = psC.tile(
                                    [P, 512], f32, tag=f"ps{ob}{ch % 2}"
                                )
                                nc.tensor.matmul(
                                    ps[:],
                                    wpT[:, 0, ob * P : (ob + 1) * P],
                                    x0[:, sl],
                                    start=True,
                                    stop=False,
                                )
                                nc.tensor.matmul(
                                    ps[:],
                                    wpT[:, 1, ob * P : (ob + 1) * P],
                                    x1[:, sl],
                                    start=False,
                                    stop=True,
                                )
                                # ELU(z+b) = max(z+b, min(exp(z+b)-1, 0))
                                e = ep.tile([P, 512], bf16, tag=f"e{ob}{ch % 2}")
                                nc.scalar.activation(
                                    e[:], ps[:], AF.Exp,
                                    bias=b_sb[:, ob : ob + 1],
                                )
                                nc.vector._custom_dve(
                                    elu_op,
                                    out=yo[:, sl],
                                    in0=ps[:],
                                    in1=e[:],
                                    s0=b_sb[:, ob : ob + 1],
                                )
                        nc.sync.dma_start(yt0[:, t * CT : (t + 1) * CT], y0[:])
                        nc.sync.dma_start(yt1[:, t * CT : (t + 1) * CT], y1[:])
    nc.compile()
    return nc


@functools.lru_cache(maxsize=4)
def _built_a(repeat=1):
    return build_a(repeat=repeat)


@functools.lru_cache(maxsize=4)
def _built_c(repeat=1):
    return build_c(repeat=repeat)


def _pjrt_fn(nc, ncores=NCORES):
    """Compile a bass module into a jitted 8-core shard_map callable.
    Returns (fn, in_names, out_names, out_avals, mesh)."""
    import jax
    from jax.experimental.shard_map import shard_map
    from jax.sharding import Mesh, PartitionSpec

    from concourse import mybir
    from concourse.bass2jax import (
        _bass_exec_p,
        install_neuronx_cc_hook,
        partition_id_tensor,
    )

    install_neuronx_cc_hook()
    partition_name = nc.partition_id_tensor.name if nc.partition_id_tensor else None
    in_names, out_names, out_avals = [], [], []
    for alloc in nc.m.functions[0].allocations:
        if not isinstance(alloc, mybir.MemoryLocationSet):
            continue
        name = alloc.memorylocations[0].name
        if alloc.kind == "ExternalInput":
            if name != partition_name:
                in_names.append(name)
        elif alloc.kind == "ExternalOutput":
            out_names.append(name)
            out_avals.append(
                jax.core.ShapedArray(
                    tuple(alloc.tensor_shape), mybir.dt.np(alloc.dtype)
                )
            )
    n_params = len(in_names)
    all_in_names = list(in_names) + list(out_names)
    if partition_name is not None:
        all_in_names.append(partition_name)

    def _body(*args):
        operands = list(args)
        if partition_name is not None:
            operands.append(partition_id_tensor())
        outs = _bass_exec_p.bind(
            *operands,
            out_avals=tuple(out_avals),
            in_names=tuple(all_in_names),
            out_names=tuple(out_names),
            lowering_input_output_aliases=(),
            sim_require_finite=True,
            sim_require_nnan=True,
            nc=nc,
        )
        return tuple(outs)

    devices = jax.devices()[:ncores]
    mesh = Mesh(np.asarray(devices), ("core",))
    spec = PartitionSpec("core")
    fn = jax.jit(
        shard_map(
            _body,
            mesh=mesh,
            in_specs=(spec,) * (n_params + len(out_names)),
            out_specs=(spec,) * len(out_names),
            check_rep=False,
        ),
        keep_unused=True,
    )
    return fn, in_names, out_names, out_avals, mesh


def _sharding():
    import jax
    from jax.sharding import Mesh, NamedSharding, PartitionSpec

    devices = jax.devices()[:NCORES]
    mesh = Mesh(np.asarray(devices), ("core",))
    return NamedSharding(mesh, PartitionSpec("core"))


def _zeros_for(out_avals):
    return [
        np.zeros((NCORES * av.shape[0], *av.shape[1:]), av.dtype) for av in out_avals
    ]


def _finalize_stats(st_host, gamma, beta):
    """st_host: [8*128, 4] partial sums -> sg [128, 4] = [s_h0|s_h1|t_h0|t_h1]."""
    st = st_host.reshape(NCORES, P, 4).sum(axis=0, dtype=np.float64)
    sums = np.concatenate([st[:, 0], st[:, 1]])  # [256]
    sqs = np.concatenate([st[:, 2], st[:, 3]])
    mean = sums / N_TOTAL
    var = sqs / N_TOTAL - mean * mean
    s = gamma.astype(np.float64) * (1.0 / np.sqrt(var + EPS))
    tt = beta.astype(np.float64) - mean * s
    sg = np.stack(
        [s[0:P], s[P:F], tt[0:P], tt[P:F]], axis=1
    ).astype(np.float32)
    return np.ascontiguousarray(sg)


def kernel(x, gamma, beta, W):
    import jax

    x = np.ascontiguousarray(np.asarray(x), dtype=np.float32)
    gamma = np.asarray(gamma, dtype=np.float32)
    beta = np.asarray(beta, dtype=np.float32)
    W = np.asarray(W, dtype=np.float32)
    assert x.shape == (N_TOTAL, F), x.shape

    sharding = _sharding()

    # ---- NEFF A: transpose-staging + stats
    nc_a = _built_a()
    fn_a, in_a, out_a, av_a, _ = _pjrt_fn(nc_a)
    assert in_a == ["x"], in_a
    x_dev = jax.device_put(x, sharding)
    outs_a = fn_a(x_dev, *[jax.device_put(z, sharding) for z in _zeros_for(av_a)])
    outs_a = dict(zip(out_a, outs_a))

    # ---- host: reduce partial stats (16 KB) and finalize scale/bias
    sg = _finalize_stats(np.asarray(outs_a["st"]), gamma, beta)
    sg_rep = np.ascontiguousarray(np.broadcast_to(sg, (NCORES, P, 4))).reshape(
        NCORES * P, 4
    )

    # ---- NEFF C: matmul + ELU (staging stays on device)
    nc_c = _built_c()
    fn_c, in_c, out_c, av_c, _ = _pjrt_fn(nc_c)
    host_ins = {
        "wt": np.concatenate([np.ascontiguousarray(W.T)] * NCORES, axis=0),
        "sg": sg_rep,
    }
    args_c = []
    for nm in in_c:
        if nm in ("xt0", "xt1"):
            args_c.append(outs_a[nm])
        else:
            args_c.append(jax.device_put(host_ins[nm], sharding))
    outs_c = fn_c(*args_c, *[jax.device_put(z, sharding) for z in _zeros_for(av_c)])
    outs_c = dict(zip(out_c, outs_c))

    # ---- host: un-transpose y^T halves, undo the (j,p) column permutation
    # from phase A's tiling (col t*1024 + j*128 + p holds row t*1024 + p*8 + j),
    # and upcast to f32 (jax cpu, threaded)
    T = N_SHARD // (P * RT)
    yt0 = np.asarray(outs_c["yt0"]).reshape(NCORES, P, N_SHARD)
    yt1 = np.asarray(outs_c["yt1"]).reshape(NCORES, P, N_SHARD)
    cpu = jax.devices("cpu")[0]
    with jax.default_device(cpu):
        yt = jax.numpy.concatenate(
            [jax.numpy.asarray(yt0), jax.numpy.asarray(yt1)], axis=1
        )  # [8, 256, N_SHARD] bf16, cols = (t, j, p)
        yt = yt.reshape(NCORES, F, T, RT, P)
        y = jax.numpy.transpose(yt, (0, 2, 4, 3, 1)).astype(jax.numpy.float32)
        y = np.asarray(y).reshape(N_TOTAL, F)
    return np.ascontiguousarray(y)


if __name__ == "__main__":
    nca = build_a()
    ncc = build_c()
    print("built OK")


# revision 11
# speedup vs baseline: 1.6680x; 1.6680x over previous
"""Two-NEFF Trainium2 kernel for fused BatchNorm1d(train) -> Linear -> ELU.

  y = ELU( ((x - mean) * gamma.rsqrt(var+eps) + beta) @ W.T )

Data-parallel over 8 cores (131072 rows each); the 2KB cross-core stat
reduction plus the 256-element stat finalization run on the HOST between
two NEFF launches (measured cheaper than an on-device collective).

  NEFF A (per core): stream x (f32), ACT-downcast to bf16, PE-transpose
      128x128 bf16 blocks (1 cyc/row) into PSUM, DVE evacuation with fused
      accum_out (per-feature sum) + DVE square with fused accum_out (sum of
      squares), stage x^T as two [128, N] bf16 DRAM tensors with plain
      2KB-line DMAs. Staged columns are (j,p)-permuted within each
      1024-column tile (input tiling is n = t*1024 + p*8 + j); phase C is
      column-independent and the host un-permutes for free.
  host: reduce the 8x[128,4] partial stats, finalize
      s = gamma*rsqrt(var+eps), t = beta - mean*s  (float64).
  NEFF C (per core): fold s into W (per-partition scale of W.T), bias
      b' = t @ W.T via tiny f32 matmuls; stream x^T bf16 with contiguous
      DMAs, W-stationary bf16 matmuls -> y^T in PSUM (output features on
      partitions, so b' is a per-partition bias); ELU via
      ELU(z) = max(z, min(exp(z)-1, 0)): ACT Exp (fused bias) + ONE fused
      custom-DVE op (ELU_TAIL_ANT, registered below); write y^T bf16.
  host: transpose y^T -> y and upcast to f32 (jax cpu).
"""

import functools
import sys

import numpy as np

if "/opt/trn_rl_repo" not in sys.path:
    sys.path.insert(0, "/opt/trn_rl_repo")

N_TOTAL = 1048576
F = 256
NCORES = 8
N_SHARD = N_TOTAL // NCORES
P = 128
RT = 8  # row-blocks per phase-A tile (1024 rows)
EPS = 1e-5


def _register_elu_tail():
    """Register the fused ELU-tail custom DVE op (idempotent).

    out = max(in0 + s0, min(in1 - 1, 0))   [in0 = z-b from PSUM, in1 = exp(z),
                                            s0 = per-partition bias b]
    """
    import concourse.dve_ops as DO
    from concourse.dve_spec import C0, One, Spec, Src0, Src1, Zero, lower, maxx, minn
    from concourse.dve_uop import DveOpSpec

    name = "ELU_TAIL_ANT"
    if name in DO._SUB_OPCODE_FOR_NAME:
        return next(op for op in DO.OPS if op.name == name)
    spec = Spec(
        body=maxx(Src0 + C0, minn(Src1 - One, Zero)),
        reference=lambda in0, in1, s0, s1, imm2: np.maximum(
            in0 + s0, np.minimum(in1 - 1.0, 0.0)
        ),
    )
    shas = {}
    for ver in ("v3", "v4"):
        ds = DveOpSpec(
            name=name, opcode=0, uops=lower(spec, ver=ver),
            rd1_en=DO.has_src1(spec),
        )
        shas[ver] = ds.sha(ver)
    op = DO.DveOp(name, spec, subdim=False, uops_sha=shas)
    DO.OPS.append(op)
    DO.CUSTOM_DVE_SPECS[op.name] = op.spec
    DO._SUB_OPCODE_FOR_NAME[op.name] = DO._CUSTOM_DVE_ROW_BASE + len(DO.OPS) - 1
    return op


def _bass(ncores):
    from concourse import bacc

    return bacc.Bacc(
        "TRN2", target_bir_lowering=False, debug=False, num_devices=ncores
    )


def build_a(n_shard=N_SHARD, ncores=NCORES, rt=RT, repeat=1):
    """Phase A: bf16 transpose-staging + per-feature stats.

    Outputs: xt0, xt1 [P, n_shard] bf16 (x^T feature halves),
             st [P, 4] f32 = [sum_h0 | sum_h1 | sumsq_h0 | sumsq_h1].
    """
    import concourse.tile as tile
    from concourse import mybir

    f32 = mybir.dt.float32
    bf16 = mybir.dt.bfloat16
    AF = mybir.ActivationFunctionType
    OP = mybir.AluOpType

    nc = _bass(ncores)
    x = nc.dram_tensor("x", [n_shard, F], f32, kind="ExternalInput").ap()
    xt0 = nc.dram_tensor("xt0", [P, n_shard], bf16, kind="ExternalOutput").ap()
    xt1 = nc.dram_tensor("xt1", [P, n_shard], bf16, kind="ExternalOutput").ap()
    st = nc.dram_tensor("st", [P, 4], f32, kind="ExternalOutput").ap()

    R = P * rt  # rows per tile
    T = n_shard // R

    with tile.TileContext(nc) as tc:
        with tc.tile_pool(name="wp", bufs=1) as wp:
            # identity for PE transposes: ident[i,j] = (j == i)
            col = wp.tile([P, P], f32)
            nc.gpsimd.iota(col[:], pattern=[[1, P]], base=0, channel_multiplier=0,
                           allow_small_or_imprecise_dtypes=True)
            row = wp.tile([P, 1], f32)
            nc.gpsimd.iota(row[:], pattern=[[0, 1]], base=0, channel_multiplier=1,
                           allow_small_or_imprecise_dtypes=True)
            ident = wp.tile([P, P], bf16)
            nc.vector.tensor_scalar(ident[:], col[:], row[:], None, OP.is_equal)

            for _rep in range(repeat):
                with tc.tile_pool(name="ac", bufs=1) as ac:
                    sumc0 = ac.tile([P, T], f32)
                    sumc1 = ac.tile([P, T], f32)
                    sqc0 = ac.tile([P, T], f32)
                    sqc1 = ac.tile([P, T], f32)
                    with tc.tile_pool(name="sa", bufs=3) as sa, tc.tile_pool(
                        name="psA", bufs=2, space="PSUM"
                    ) as psA:
                        xv = x.rearrange("(t p j) f -> t p j f", p=P, j=rt)
                        for t in range(T):
                            xt = sa.tile([P, rt, F], f32, tag="xt")
                            nc.sync.dma_start(xt[:], xv[t])
                            xb = sa.tile([P, rt, F], bf16, tag="xb")
                            nc.vector.tensor_copy(xb[:], xt[:])
                            ps0 = psA.tile([P, rt, P], bf16, tag="ps0")
                            ps1 = psA.tile([P, rt, P], bf16, tag="ps1")
                            for j in range(rt):
                                nc.tensor.transpose(
                                    ps0[:, j, :], xb[:, j, 0:P], ident[:]
                                )
                                nc.tensor.transpose(
                                    ps1[:, j, :], xb[:, j, P:F], ident[:]
                                )
                            # evacuate PSUM -> SBUF (ACT) with fused column sums
                            xs0 = sa.tile([P, rt * P], bf16, tag="xs0")
                            nc.scalar.activation(
                                xs0[:], ps0[:].rearrange("p j c -> p (j c)"),
                                AF.Identity, accum_out=sumc0[:, t : t + 1],
                            )
                            xs1 = sa.tile([P, rt * P], bf16, tag="xs1")
                            nc.scalar.activation(
                                xs1[:], ps1[:].rearrange("p j c -> p (j c)"),
                                AF.Identity, accum_out=sumc1[:, t : t + 1],
                            )
                            # sum of squares on DVE with fused accumulate
                            sq0 = sa.tile([P, rt * P], bf16, tag="sq0")
                            nc.vector.scalar_tensor_tensor(
                                sq0[:], xs0[:], 0.0, xs0[:], OP.add, OP.mult,
                                accum_out=sqc0[:, t : t + 1],
                            )
                            sq1 = sa.tile([P, rt * P], bf16, tag="sq1")
                            nc.vector.scalar_tensor_tensor(
                                sq1[:], xs1[:], 0.0, xs1[:], OP.add, OP.mult,
                                accum_out=sqc1[:, t : t + 1],
                            )
                            nc.sync.dma_start(xt0[:, t * R : (t + 1) * R], xs0[:])
                            nc.sync.dma_start(xt1[:, t * R : (t + 1) * R], xs1[:])
                    st_sb = ac.tile([P, 4], f32)
                    nc.vector.tensor_reduce(
                        st_sb[:, 0:1], sumc0[:], mybir.AxisListType.X, OP.add
                    )
                    nc.vector.tensor_reduce(
                        st_sb[:, 1:2], sumc1[:], mybir.AxisListType.X, OP.add
                    )
                    nc.vector.tensor_reduce(
                        st_sb[:, 2:3], sqc0[:], mybir.AxisListType.X, OP.add
                    )
                    nc.vector.tensor_reduce(
                        st_sb[:, 3:4], sqc1[:], mybir.AxisListType.X, OP.add
                    )
                    nc.sync.dma_start(st, st_sb[:])
    nc.compile()
    return nc


def build_c(n_shard=N_SHARD, ncores=NCORES, repeat=1):
    """Phase C: y^T = ELU(Wp @ x^T + b') with W stationary on the PE.

    Inputs: xt0, xt1 [P, n_shard] bf16; wt [F, F] f32 (= W.T);
            sg [P, 4] f32 = [s_h0 | s_h1 | t_h0 | t_h1] (host-finalized).
    Outputs: yt0, yt1 [P, n_shard] bf16 (y^T output-feature halves).
    """
    import concourse.tile as tile
    from concourse import mybir

    elu_op = _register_elu_tail()

    f32 = mybir.dt.float32
    bf16 = mybir.dt.bfloat16
    AF = mybir.ActivationFunctionType
    OP = mybir.AluOpType

    nc = _bass(ncores)
    xt0 = nc.dram_tensor("xt0", [P, n_shard], bf16, kind="ExternalInput").ap()
    xt1 = nc.dram_tensor("xt1", [P, n_shard], bf16, kind="ExternalInput").ap()
    wt = nc.dram_tensor("wt", [F, F], f32, kind="ExternalInput").ap()
    sg = nc.dram_tensor("sg", [P, 4], f32, kind="ExternalInput").ap()
    yt0 = nc.dram_tensor("yt0", [P, n_shard], bf16, kind="ExternalOutput").ap()
    yt1 = nc.dram_tensor("yt1", [P, n_shard], bf16, kind="ExternalOutput").ap()

    CT = 2048  # columns (rows of x) per DMA tile
    ST = 1024  # columns per PE sweep (one stationary-load group)
    T = n_shard // CT

    with tile.TileContext(nc) as tc:
        with tc.tile_pool(name="wp", bufs=1) as wp:
            wt_sb = wp.tile([P, 2, F], f32)
            nc.sync.dma_start(wt_sb[:], wt.rearrange("(c p) f -> p c f", p=P))
            sg_sb = wp.tile([P, 4], f32)
            nc.sync.dma_start(sg_sb[:], sg)

            for _rep in range(repeat):
                with tc.tile_pool(name="pb", bufs=1, space="PSUM") as psB:
                    # Wp^T[f, o] = W^T[f, o] * s[f]  (per-partition scale)
                    wpT = wp.tile([P, 2, F], bf16)
                    for c in range(2):
                        nc.vector.tensor_scalar(
                            wpT[:, c, :], wt_sb[:, c, :], sg_sb[:, c : c + 1],
                            None, OP.mult,
                        )
                    # b'[o] = sum_f t[f] * W^T[f, o], as two [128,1] columns
                    bps = psB.tile([P, 2], f32, tag="bps")
                    for ob in range(2):
                        for c in range(2):
                            nc.tensor.matmul(
                                bps[:, ob : ob + 1],
                                wt_sb[:, c, ob * P : (ob + 1) * P],
                                sg_sb[:, 2 + c : 3 + c],
                                start=(c == 0),
                                stop=(c == 1),
                            )
                    b_sb = wp.tile([P, 2], f32)
                    nc.vector.tensor_copy(b_sb[:], bps[:])

                with tc.tile_pool(name="cp", bufs=2) as cp, tc.tile_pool(
                    name="ep", bufs=2
                ) as ep, tc.tile_pool(name="psC", bufs=2, space="PSUM") as psC:
                    for t in range(T):
                        x0 = cp.tile([P, CT], bf16, tag="x0")
                        nc.sync.dma_start(x0[:], xt0[:, t * CT : (t + 1) * CT])
                        x1 = cp.tile([P, CT], bf16, tag="x1")
                        nc.sync.dma_start(x1[:], xt1[:, t * CT : (t + 1) * CT])
                        y0 = cp.tile([P, CT], bf16, tag="y0")
                        y1 = cp.tile([P, CT], bf16, tag="y1")
                        for s in range(CT // ST):
                            # one sweep: hold each W stationary over ST cols
                            pss = {}
                            for ob in range(2):
                                for ch in range(ST // 512):
                                    pss[ob, ch] = psC.tile(
                                        [P, 512], f32, tag=f"ps{ob}{ch}",
                                        name=f"ps{ob}{ch}",
                                    )
                            for ob in range(2):
                                for h, xh in ((0, x0), (1, x1)):
                                    for ch in range(ST // 512):
                                        sl = slice(
                                            s * ST + ch * 512,
                                            s * ST + (ch + 1) * 512,
                                        )
                                        nc.tensor.matmul(
                                            pss[ob, ch][:],
                                            wpT[:, h, ob * P : (ob + 1) * P],
                                            xh[:, sl],
                                            start=(h == 0),
                                            stop=(h == 1),
                                        )
                            # ELU(z+b) = max(z+b, min(exp(z+b)-1, 0))
                            for ob, yo in ((0, y0), (1, y1)):
                                for ch in range(ST // 512):
                                    sl = slice(
                                        s * ST + ch * 512,
                                        s * ST + (ch + 1) * 512,
                                    )
                                    ps = pss[ob, ch]
                                    e = ep.tile(
                                        [P, 512], bf16, tag=f"e{ob}{ch}"
                                    )
                                    nc.scalar.activation(
                                        e[:], ps[:], AF.Exp,
                                        bias=b_sb[:, ob : ob + 1],
                                    )
                                    nc.vector._custom_dve(
                                        elu_op,
                                        out=yo[:, sl],
                                        in0=ps[:],
                                        in1=e[:],
                                        s0=b_sb[:, ob : ob + 1],
                                    )
                        nc.sync.dma_start(yt0[:, t * CT : (t + 1) * CT], y0[:])
                        nc.sync.dma_start(yt1[:, t * CT : (t + 1) * CT], y1[:])
    nc.compile()
    return nc


@functools.lru_cache(maxsize=4)
def _built_a(repeat=1):
    return build_a(repeat=repeat)


@functools.lru_cache(maxsize=4)
def _built_c(repeat=1):
    return build_c(repeat=repeat)


def _pjrt_fn(nc, ncores=NCORES):
    """Compile a bass module into a jitted 8-core shard_map callable.
    Returns (fn, in_names, out_names, out_avals, mesh)."""
    import jax
    from jax.experimental.shard_map import shard_map
    from jax.sharding import Mesh, PartitionSpec

    from concourse import mybir
    from concourse.bass2jax import (
        _bass_exec_p,
        install_neuronx_cc_hook,
        partition_id_tensor,
    )

    install_neuronx_cc_hook()
    partition_name = nc.partition_id_tensor.name if nc.partition_id_tensor else None
    in_names, out_names, out_avals = [], [], []
    for alloc in nc.m.functions[0].allocations:
        if not isinstance(alloc, mybir.MemoryLocationSet):
            continue
        name = alloc.memorylocations[0].name
        if alloc.kind == "ExternalInput":
            if name != partition_name:
                in_names.append(name)
        elif alloc.kind == "ExternalOutput":
            out_names.append(name)
            out_avals.append(
                jax.core.ShapedArray(
                    tuple(alloc.tensor_shape), mybir.dt.np(alloc.dtype)
                )
            )
    n_params = len(in_names)
    all_in_names = list(in_names) + list(out_names)
    if partition_name is not None:
        all_in_names.append(partition_name)

    def _body(*args):
        operands = list(args)
        if partition_name is not None:
            operands.append(partition_id_tensor())
        outs = _bass_exec_p.bind(
            *operands,
            out_avals=tuple(out_avals),
            in_names=tuple(all_in_names),
            out_names=tuple(out_names),
            lowering_input_output_aliases=(),
            sim_require_finite=True,
            sim_require_nnan=True,
            nc=nc,
        )
        return tuple(outs)

    devices = jax.devices()[:ncores]
    mesh = Mesh(np.asarray(devices), ("core",))
    spec = PartitionSpec("core")
    fn = jax.jit(
        shard_map(
            _body,
            mesh=mesh,
            in_specs=(spec,) * (n_params + len(out_names)),
            out_specs=(spec,) * len(out_names),
            check_rep=False,
        ),
        keep_unused=True,
    )
    return fn, in_names, out_names, out_avals, mesh


def _sharding():
    import jax
    from jax.sharding import Mesh, NamedSharding, PartitionSpec

    devices = jax.devices()[:NCORES]
    mesh = Mesh(np.asarray(devices), ("core",))
    return NamedSharding(mesh, PartitionSpec("core"))


def _zeros_for(out_avals):
    return [
        np.zeros((NCORES * av.shape[0], *av.shape[1:]), av.dtype) for av in out_avals
    ]


def _finalize_stats(st_host, gamma, beta):
    """st_host: [8*128, 4] partial sums -> sg [128, 4] = [s_h0|s_h1|t_h0|t_h1]."""
    st = st_host.reshape(NCORES, P, 4).sum(axis=0, dtype=np.float64)
    sums = np.concatenate([st[:, 0], st[:, 1]])  # [256]
    sqs = np.concatenate([st[:, 2], st[:, 3]])
    mean = sums / N_TOTAL
    var = sqs / N_TOTAL - mean * mean
    s = gamma.astype(np.float64) * (1.0 / np.sqrt(var + EPS))
    tt = beta.astype(np.float64) - mean * s
    sg = np.stack(
        [s[0:P], s[P:F], tt[0:P], tt[P:F]], axis=1
    ).astype(np.float32)
    return np.ascontiguousarray(sg)


def kernel(x, gamma, beta, W):
    import jax

    x = np.ascontiguousarray(np.asarray(x), dtype=np.float32)
    gamma = np.asarray(gamma, dtype=np.float32)
    beta = np.asarray(beta, dtype=np.float32)
    W = np.asarray(W, dtype=np.float32)
    assert x.shape == (N_TOTAL, F), x.shape

    sharding = _sharding()

    # ---- NEFF A: transpose-staging + stats
    nc_a = _built_a()
    fn_a, in_a, out_a, av_a, _ = _pjrt_fn(nc_a)
    assert in_a == ["x"], in_a
    x_dev = jax.device_put(x, sharding)
    outs_a = fn_a(x_dev, *[jax.device_put(z, sharding) for z in _zeros_for(av_a)])
    outs_a = dict(zip(out_a, outs_a))

    # ---- host: reduce partial stats (16 KB) and finalize scale/bias
    sg = _finalize_stats(np.asarray(outs_a["st"]), gamma, beta)
    sg_rep = np.ascontiguousarray(np.broadcast_to(sg, (NCORES, P, 4))).reshape(
        NCORES * P, 4
    )

    # ---- NEFF C: matmul + ELU (staging stays on device)
    nc_c = _built_c()
    fn_c, in_c, out_c, av_c, _ = _pjrt_fn(nc_c)
    host_ins = {
        "wt": np.concatenate([np.ascontiguousarray(W.T)] * NCORES, axis=0),
        "sg": sg_rep,
    }
    args_c = []
    for nm in in_c:
        if nm in ("xt0", "xt1"):
            args_c.append(outs_a[nm])
        else:
            args_c.append(jax.device_put(host_ins[nm], sharding))
    outs_c = fn_c(*args_c, *[jax.device_put(z, sharding) for z in _zeros_for(av_c)])
    outs_c = dict(zip(out_c, outs_c))

    # ---- host: un-transpose y^T halves, undo the (j,p) column permutation
    # from phase A's tiling (col t*1024 + j*128 + p holds row t*1024 + p*8 + j),
    # and upcast to f32 (jax cpu, threaded)
    T = N_SHARD // (P * RT)
    yt0 = np.asarray(outs_c["yt0"]).reshape(NCORES, P, N_SHARD)
    yt1 = np.asarray(outs_c["yt1"]).reshape(NCORES, P, N_SHARD)
    cpu = jax.devices("cpu")[0]
    with jax.default_device(cpu):
        yt = jax.numpy.concatenate(
            [jax.numpy.asarray(yt0), jax.numpy.asarray(yt1)], axis=1
        )  # [8, 256, N_SHARD] bf16, cols = (t, j, p)
        yt = yt.reshape(NCORES, F, T, RT, P)
        y = jax.numpy.transpose(yt, (0, 2, 4, 3, 1)).astype(jax.numpy.float32)
        y = np.asarray(y).reshape(N_TOTAL, F)
    return np.ascontiguousarray(y)


if __name__ == "__main__":
    nca = build_a()
    ncc = build_c()
    print("built OK")


# revision 12
# speedup vs baseline: 7.9391x; 4.7596x over previous
"""Two-NEFF Trainium2 kernel for fused BatchNorm1d(train) -> Linear -> ELU.

  y = ELU( ((x - mean) * gamma.rsqrt(var+eps) + beta) @ W.T )

Data-parallel over 8 cores (131072 rows each); the 2KB cross-core stat
reduction plus the 256-element stat finalization run on the HOST between
two NEFF launches (measured cheaper than an on-device collective).

  NEFF A (per core): stream x (f32), ACT-downcast to bf16, PE-transpose
      128x128 bf16 blocks (1 cyc/row) into PSUM, DVE evacuation with fused
      accum_out (per-feature sum) + DVE square with fused accum_out (sum of
      squares), stage x^T as two [128, N] bf16 DRAM tensors with plain
      2KB-line DMAs. Staged columns are (j,p)-permuted within each
      1024-column tile (input tiling is n = t*1024 + p*8 + j); phase C is
      column-independent and the host un-permutes for free.
  host: reduce the 8x[128,4] partial stats, finalize
      s = gamma*rsqrt(var+eps), t = beta - mean*s  (float64).
  NEFF C (per core): fold s into W (per-partition scale of W.T), bias
      b' = t @ W.T via tiny f32 matmuls; stream x^T bf16 with contiguous
      DMAs, W-stationary bf16 matmuls -> y^T in PSUM (output features on
      partitions, so b' is a per-partition bias); ELU via
      ELU(z) = max(z, min(exp(z)-1, 0)): ACT Exp (fused bias) + ONE fused
      custom-DVE op (ELU_TAIL_ANT, registered below); write y^T bf16.
  host: transpose y^T -> y and upcast to f32 (jax cpu).
"""

import functools
import sys

import numpy as np

if "/opt/trn_rl_repo" not in sys.path:
    sys.path.insert(0, "/opt/trn_rl_repo")

N_TOTAL = 1048576
F = 256
NCORES = 8
N_SHARD = N_TOTAL // NCORES
P = 128
RT = 8  # row-blocks per phase-A tile (1024 rows)
EPS = 1e-5


def _register_elu_tail():
    """Register the fused ELU-tail custom DVE op (idempotent).

    out = max(in0 + s0, min(in1 - 1, 0))   [in0 = z-b from PSUM, in1 = exp(z),
                                            s0 = per-partition bias b]
    """
    import concourse.dve_ops as DO
    from concourse.dve_spec import C0, One, Spec, Src0, Src1, Zero, lower, maxx, minn
    from concourse.dve_uop import DveOpSpec

    name = "ELU_TAIL_ANT"
    if name in DO._SUB_OPCODE_FOR_NAME:
        return next(op for op in DO.OPS if op.name == name)
    spec = Spec(
        body=maxx(Src0 + C0, minn(Src1 - One, Zero)),
        reference=lambda in0, in1, s0, s1, imm2: np.maximum(
            in0 + s0, np.minimum(in1 - 1.0, 0.0)
        ),
    )
    shas = {}
    for ver in ("v3", "v4"):
        ds = DveOpSpec(
            name=name, opcode=0, uops=lower(spec, ver=ver),
            rd1_en=DO.has_src1(spec),
        )
        shas[ver] = ds.sha(ver)
    op = DO.DveOp(name, spec, subdim=False, uops_sha=shas)
    DO.OPS.append(op)
    DO.CUSTOM_DVE_SPECS[op.name] = op.spec
    DO._SUB_OPCODE_FOR_NAME[op.name] = DO._CUSTOM_DVE_ROW_BASE + len(DO.OPS) - 1
    return op


def _bass(ncores):
    from concourse import bacc

    return bacc.Bacc(
        "TRN2", target_bir_lowering=False, debug=False, num_devices=ncores
    )


def build_a(n_shard=N_SHARD, ncores=NCORES, rt=RT, repeat=1):
    """Phase A: bf16 transpose-staging + per-feature stats.

    Outputs: xt0, xt1 [P, n_shard] bf16 (x^T feature halves),
             st [P, 4] f32 = [sum_h0 | sum_h1 | sumsq_h0 | sumsq_h1].
    """
    import concourse.tile as tile
    from concourse import mybir

    f32 = mybir.dt.float32
    bf16 = mybir.dt.bfloat16
    AF = mybir.ActivationFunctionType
    OP = mybir.AluOpType

    nc = _bass(ncores)
    x = nc.dram_tensor("x", [n_shard, F], f32, kind="ExternalInput").ap()
    xt0 = nc.dram_tensor("xt0", [P, n_shard], bf16, kind="ExternalOutput").ap()
    xt1 = nc.dram_tensor("xt1", [P, n_shard], bf16, kind="ExternalOutput").ap()
    st = nc.dram_tensor("st", [P, 4], f32, kind="ExternalOutput").ap()

    R = P * rt  # rows per tile
    T = n_shard // R

    with tile.TileContext(nc) as tc:
        with tc.tile_pool(name="wp", bufs=1) as wp:
            # identity for PE transposes: ident[i,j] = (j == i)
            col = wp.tile([P, P], f32)
            nc.gpsimd.iota(col[:], pattern=[[1, P]], base=0, channel_multiplier=0,
                           allow_small_or_imprecise_dtypes=True)
            row = wp.tile([P, 1], f32)
            nc.gpsimd.iota(row[:], pattern=[[0, 1]], base=0, channel_multiplier=1,
                           allow_small_or_imprecise_dtypes=True)
            ident = wp.tile([P, P], bf16)
            nc.vector.tensor_scalar(ident[:], col[:], row[:], None, OP.is_equal)

            for _rep in range(repeat):
                with tc.tile_pool(name="ac", bufs=1) as ac:
                    sumc0 = ac.tile([P, T], f32)
                    sumc1 = ac.tile([P, T], f32)
                    sqc0 = ac.tile([P, T], f32)
                    sqc1 = ac.tile([P, T], f32)
                    with tc.tile_pool(name="sa", bufs=3) as sa, tc.tile_pool(
                        name="psA", bufs=2, space="PSUM"
                    ) as psA:
                        xv = x.rearrange("(t p j) f -> t p j f", p=P, j=rt)
                        for t in range(T):
                            xt = sa.tile([P, rt, F], f32, tag="xt")
                            nc.sync.dma_start(xt[:], xv[t])
                            xb = sa.tile([P, rt, F], bf16, tag="xb")
                            nc.vector.tensor_copy(xb[:], xt[:])
                            ps0 = psA.tile([P, rt, P], bf16, tag="ps0")
                            ps1 = psA.tile([P, rt, P], bf16, tag="ps1")
                            for j in range(rt):
                                nc.tensor.transpose(
                                    ps0[:, j, :], xb[:, j, 0:P], ident[:]
                                )
                                nc.tensor.transpose(
                                    ps1[:, j, :], xb[:, j, P:F], ident[:]
                                )
                            # evacuate PSUM -> SBUF (ACT) with fused column sums
                            xs0 = sa.tile([P, rt * P], bf16, tag="xs0")
                            nc.scalar.activation(
                                xs0[:], ps0[:].rearrange("p j c -> p (j c)"),
                                AF.Identity, accum_out=sumc0[:, t : t + 1],
                            )
                            xs1 = sa.tile([P, rt * P], bf16, tag="xs1")
                            nc.scalar.activation(
                                xs1[:], ps1[:].rearrange("p j c -> p (j c)"),
                                AF.Identity, accum_out=sumc1[:, t : t + 1],
                            )
                            # sum of squares on DVE with fused accumulate
                            sq0 = sa.tile([P, rt * P], bf16, tag="sq0")
                            nc.vector.scalar_tensor_tensor(
                                sq0[:], xs0[:], 0.0, xs0[:], OP.add, OP.mult,
                                accum_out=sqc0[:, t : t + 1],
                            )
                            sq1 = sa.tile([P, rt * P], bf16, tag="sq1")
                            nc.vector.scalar_tensor_tensor(
                                sq1[:], xs1[:], 0.0, xs1[:], OP.add, OP.mult,
                                accum_out=sqc1[:, t : t + 1],
                            )
                            nc.sync.dma_start(xt0[:, t * R : (t + 1) * R], xs0[:])
                            nc.sync.dma_start(xt1[:, t * R : (t + 1) * R], xs1[:])
                    st_sb = ac.tile([P, 4], f32)
                    nc.vector.tensor_reduce(
                        st_sb[:, 0:1], sumc0[:], mybir.AxisListType.X, OP.add
                    )
                    nc.vector.tensor_reduce(
                        st_sb[:, 1:2], sumc1[:], mybir.AxisListType.X, OP.add
                    )
                    nc.vector.tensor_reduce(
                        st_sb[:, 2:3], sqc0[:], mybir.AxisListType.X, OP.add
                    )
                    nc.vector.tensor_reduce(
                        st_sb[:, 3:4], sqc1[:], mybir.AxisListType.X, OP.add
                    )
                    nc.sync.dma_start(st, st_sb[:])
    nc.compile()
    return nc


def build_c(n_shard=N_SHARD, ncores=NCORES, repeat=1):
    """Phase C: y^T = ELU(Wp @ x^T + b') with W stationary on the PE.

    Inputs: xt0, xt1 [P, n_shard] bf16; wt [F, F] f32 (= W.T);
            sg [P, 4] f32 = [s_h0 | s_h1 | t_h0 | t_h1] (host-finalized).
    Outputs: yt0, yt1 [P, n_shard] bf16 (y^T output-feature halves).
    """
    import concourse.tile as tile
    from concourse import mybir

    elu_op = _register_elu_tail()

    f32 = mybir.dt.float32
    bf16 = mybir.dt.bfloat16
    AF = mybir.ActivationFunctionType
    OP = mybir.AluOpType

    nc = _bass(ncores)
    xt0 = nc.dram_tensor("xt0", [P, n_shard], bf16, kind="ExternalInput").ap()
    xt1 = nc.dram_tensor("xt1", [P, n_shard], bf16, kind="ExternalInput").ap()
    wt = nc.dram_tensor("wt", [F, F], f32, kind="ExternalInput").ap()
    sg = nc.dram_tensor("sg", [P, 4], f32, kind="ExternalInput").ap()
    yt0 = nc.dram_tensor("yt0", [P, n_shard], bf16, kind="ExternalOutput").ap()
    yt1 = nc.dram_tensor("yt1", [P, n_shard], bf16, kind="ExternalOutput").ap()

    CT = 2048  # columns (rows of x) per DMA tile
    ST = 1024  # columns per PE sweep (one stationary-load group)
    T = n_shard // CT

    with tile.TileContext(nc) as tc:
        with tc.tile_pool(name="wp", bufs=1) as wp:
            wt_sb = wp.tile([P, 2, F], f32)
            nc.sync.dma_start(wt_sb[:], wt.rearrange("(c p) f -> p c f", p=P))
            sg_sb = wp.tile([P, 4], f32)
            nc.sync.dma_start(sg_sb[:], sg)

            for _rep in range(repeat):
                with tc.tile_pool(name="pb", bufs=1, space="PSUM") as psB:
                    # Wp^T[f, o] = W^T[f, o] * s[f]  (per-partition scale)
                    wpT = wp.tile([P, 2, F], bf16)
                    for c in range(2):
                        nc.vector.tensor_scalar(
                            wpT[:, c, :], wt_sb[:, c, :], sg_sb[:, c : c + 1],
                            None, OP.mult,
                        )
                    # b'[o] = sum_f t[f] * W^T[f, o], as two [128,1] columns
                    bps = psB.tile([P, 2], f32, tag="bps")
                    for ob in range(2):
                        for c in range(2):
                            nc.tensor.matmul(
                                bps[:, ob : ob + 1],
                                wt_sb[:, c, ob * P : (ob + 1) * P],
                                sg_sb[:, 2 + c : 3 + c],
                                start=(c == 0),
                                stop=(c == 1),
                            )
                    b_sb = wp.tile([P, 2], f32)
                    nc.vector.tensor_copy(b_sb[:], bps[:])

                with tc.tile_pool(name="cp", bufs=2) as cp, tc.tile_pool(
                    name="ep", bufs=2
                ) as ep, tc.tile_pool(name="psC", bufs=2, space="PSUM") as psC:
                    for t in range(T):
                        x0 = cp.tile([P, CT], bf16, tag="x0")
                        nc.sync.dma_start(x0[:], xt0[:, t * CT : (t + 1) * CT])
                        x1 = cp.tile([P, CT], bf16, tag="x1")
                        nc.sync.dma_start(x1[:], xt1[:, t * CT : (t + 1) * CT])
                        y0 = cp.tile([P, CT], bf16, tag="y0")
                        y1 = cp.tile([P, CT], bf16, tag="y1")
                        for s in range(CT // ST):
                            # one sweep: hold each W stationary over ST cols
                            pss = {}
                            for ob in range(2):
                                pss[ob] = psC.tile(
                                    [P, ST], f32, tag=f"ps{ob}", name=f"ps{ob}"
                                )
                            for ob in range(2):
                                for h, xh in ((0, x0), (1, x1)):
                                    for ch in range(ST // 512):
                                        sl = slice(
                                            s * ST + ch * 512,
                                            s * ST + (ch + 1) * 512,
                                        )
                                        nc.tensor.matmul(
                                            pss[ob][:, ch * 512 : (ch + 1) * 512],
                                            wpT[:, h, ob * P : (ob + 1) * P],
                                            xh[:, sl],
                                            start=(h == 0),
                                            stop=(h == 1),
                                        )
                            # ELU(z+b) = max(z+b, min(exp(z+b)-1, 0))
                            sl = slice(s * ST, (s + 1) * ST)
                            for ob, yo in ((0, y0), (1, y1)):
                                e = ep.tile([P, ST], bf16, tag=f"e{ob}")
                                nc.scalar.activation(
                                    e[:], pss[ob][:], AF.Exp,
                                    bias=b_sb[:, ob : ob + 1],
                                )
                                nc.vector._custom_dve(
                                    elu_op,
                                    out=yo[:, sl],
                                    in0=pss[ob][:],
                                    in1=e[:],
                                    s0=b_sb[:, ob : ob + 1],
                                )
                        nc.sync.dma_start(yt0[:, t * CT : (t + 1) * CT], y0[:])
                        nc.sync.dma_start(yt1[:, t * CT : (t + 1) * CT], y1[:])
    nc.compile()
    return nc


@functools.lru_cache(maxsize=4)
def _built_a(repeat=1):
    return build_a(repeat=repeat)


@functools.lru_cache(maxsize=4)
def _built_c(repeat=1):
    return build_c(repeat=repeat)


def _pjrt_fn(nc, ncores=NCORES):
    """Compile a bass module into a jitted 8-core shard_map callable.
    Returns (fn, in_names, out_names, out_avals, mesh)."""
    import jax
    from jax.experimental.shard_map import shard_map
    from jax.sharding import Mesh, PartitionSpec

    from concourse import mybir
    from concourse.bass2jax import (
        _bass_exec_p,
        install_neuronx_cc_hook,
        partition_id_tensor,
    )

    install_neuronx_cc_hook()
    partition_name = nc.partition_id_tensor.name if nc.partition_id_tensor else None
    in_names, out_names, out_avals = [], [], []
    for alloc in nc.m.functions[0].allocations:
        if not isinstance(alloc, mybir.MemoryLocationSet):
            continue
        name = alloc.memorylocations[0].name
        if alloc.kind == "ExternalInput":
            if name != partition_name:
                in_names.append(name)
        elif alloc.kind == "ExternalOutput":
            out_names.append(name)
            out_avals.append(
                jax.core.ShapedArray(
                    tuple(alloc.tensor_shape), mybir.dt.np(alloc.dtype)
                )
            )
    n_params = len(in_names)
    all_in_names = list(in_names) + list(out_names)
    if partition_name is not None:
        all_in_names.append(partition_name)

    def _body(*args):
        operands = list(args)
        if partition_name is not None:
            operands.append(partition_id_tensor())
        outs = _bass_exec_p.bind(
            *operands,
            out_avals=tuple(out_avals),
            in_names=tuple(all_in_names),
            out_names=tuple(out_names),
            lowering_input_output_aliases=(),
            sim_require_finite=True,
            sim_require_nnan=True,
            nc=nc,
        )
        return tuple(outs)

    devices = jax.devices()[:ncores]
    mesh = Mesh(np.asarray(devices), ("core",))
    spec = PartitionSpec("core")
    fn = jax.jit(
        shard_map(
            _body,
            mesh=mesh,
            in_specs=(spec,) * (n_params + len(out_names)),
            out_specs=(spec,) * len(out_names),
            check_rep=False,
        ),
        keep_unused=True,
    )
    return fn, in_names, out_names, out_avals, mesh


def _sharding():
    import jax
    from jax.sharding import Mesh, NamedSharding, PartitionSpec

    devices = jax.devices()[:NCORES]
    mesh = Mesh(np.asarray(devices), ("core",))
    return NamedSharding(mesh, PartitionSpec("core"))


def _zeros_for(out_avals):
    return [
        np.zeros((NCORES * av.shape[0], *av.shape[1:]), av.dtype) for av in out_avals
    ]


def _finalize_stats(st_host, gamma, beta):
    """st_host: [8*128, 4] partial sums -> sg [128, 4] = [s_h0|s_h1|t_h0|t_h1]."""
    st = st_host.reshape(NCORES, P, 4).sum(axis=0, dtype=np.float64)
    sums = np.concatenate([st[:, 0], st[:, 1]])  # [256]
    sqs = np.concatenate([st[:, 2], st[:, 3]])
    mean = sums / N_TOTAL
    var = sqs / N_TOTAL - mean * mean
    s = gamma.astype(np.float64) * (1.0 / np.sqrt(var + EPS))
    tt = beta.astype(np.float64) - mean * s
    sg = np.stack(
        [s[0:P], s[P:F], tt[0:P], tt[P:F]], axis=1
    ).astype(np.float32)
    return np.ascontiguousarray(sg)


def kernel(x, gamma, beta, W):
    import jax

    x = np.ascontiguousarray(np.asarray(x), dtype=np.float32)
    gamma = np.asarray(gamma, dtype=np.float32)
    beta = np.asarray(beta, dtype=np.float32)
    W = np.asarray(W, dtype=np.float32)
    assert x.shape == (N_TOTAL, F), x.shape

    sharding = _sharding()

    # ---- NEFF A: transpose-staging + stats
    nc_a = _built_a()
    fn_a, in_a, out_a, av_a, _ = _pjrt_fn(nc_a)
    assert in_a == ["x"], in_a
    x_dev = jax.device_put(x, sharding)
    outs_a = fn_a(x_dev, *[jax.device_put(z, sharding) for z in _zeros_for(av_a)])
    outs_a = dict(zip(out_a, outs_a))

    # ---- host: reduce partial stats (16 KB) and finalize scale/bias
    sg = _finalize_stats(np.asarray(outs_a["st"]), gamma, beta)
    sg_rep = np.ascontiguousarray(np.broadcast_to(sg, (NCORES, P, 4))).reshape(
        NCORES * P, 4
    )

    # ---- NEFF C: matmul + ELU (staging stays on device)
    nc_c = _built_c()
    fn_c, in_c, out_c, av_c, _ = _pjrt_fn(nc_c)
    host_ins = {
        "wt": np.concatenate([np.ascontiguousarray(W.T)] * NCORES, axis=0),
        "sg": sg_rep,
    }
    args_c = []
    for nm in in_c:
        if nm in ("xt0", "xt1"):
            args_c.append(outs_a[nm])
        else:
            args_c.append(jax.device_put(host_ins[nm], sharding))
    outs_c = fn_c(*args_c, *[jax.device_put(z, sharding) for z in _zeros_for(av_c)])
    outs_c = dict(zip(out_c, outs_c))

    # ---- host: un-transpose y^T halves, undo the (j,p) column permutation
    # from phase A's tiling (col t*1024 + j*128 + p holds row t*1024 + p*8 + j),
    # and upcast to f32 (jax cpu, threaded)
    T = N_SHARD // (P * RT)
    yt0 = np.asarray(outs_c["yt0"]).reshape(NCORES, P, N_SHARD)
    yt1 = np.asarray(outs_c["yt1"]).reshape(NCORES, P, N_SHARD)
    cpu = jax.devices("cpu")[0]
    with jax.default_device(cpu):
        yt = jax.numpy.concatenate(
            [jax.numpy.asarray(yt0), jax.numpy.asarray(yt1)], axis=1
        )  # [8, 256, N_SHARD] bf16, cols = (t, j, p)
        yt = yt.reshape(NCORES, F, T, RT, P)
        y = jax.numpy.transpose(yt, (0, 2, 4, 3, 1)).astype(jax.numpy.float32)
        y = np.asarray(y).reshape(N_TOTAL, F)
    return np.ascontiguousarray(y)


if __name__ == "__main__":
    nca = build_a()
    ncc = build_c()
    print("built OK")
